# revision 1
# baseline (speedup 1.0000x reference)
"""Weighted Pearson correlation (six fused global reductions) on 8 trn2 cores.

Sharding: data-parallel over the flat N=2^25 dimension; each core reduces its
4M-element shard to a small set of partial sums which the host combines.

Per-core engine split (targets the ~140us/core HBM roofline for the 48MiB shard):
  - DVE    : 3 fused tensor_tensor_reduce ops/tile -> products nx=n*x, ny=n*y
             plus the cancellation-sensitive sums (sum_nx, sum_ny, sum_nxy),
             all in fp32.
  - GPSIMD : 1 tensor_tensor mul/tile -> nxx = nx*x.
  - ACT    : 2 activation-Copy ops/tile with accum_out -> sum_n, sum_nxx
             (free-axis fp32 accumulation; ACT is otherwise idle).
  - PE     : fp32 "diagonal" matmuls: diag(ny_chunk^T @ y_chunk) accumulated in
             one PSUM bank across all chunks/tiles; the diagonal of the final
             128x128 block is the per-column partial of sum_ny2.
Host: gathers per-core partials (a few KB), reduces in float64, applies the
12-flop correlation formula.
"""

import numpy as np

import concourse.bass as bass
import concourse.bacc as bacc
import concourse.tile as tile
from concourse import mybir
from concourse.bass_utils import run_bass_kernel_spmd

N_TOTAL = 33554432  # 2^25
N_CORES = 8
P = 128  # SBUF partitions

# Per-core shard: 4194304 elements = T tiles of [P, F]
F = 1024
T = N_TOTAL // N_CORES // (P * F)  # 32

_F32 = mybir.dt.float32
_MULT = mybir.AluOpType.mult
_ADD = mybir.AluOpType.add
_COPY = mybir.ActivationFunctionType.Copy


def build_nc(tiles=T, free=F, in_bufs=3, prod_bufs=2, rounds=1, variant=3):
    """Build the per-core Bass program. All 8 cores run this same program on
    their own shard (inputs shaped [tiles, 128, free])."""
    f = free
    c128 = f // 128  # stationary operand is at most 128 columns

    nc = bacc.Bacc(None)
    xs = nc.dram_tensor("xs", [tiles, P, f], _F32, kind="ExternalInput")
    ys = nc.dram_tensor("ys", [tiles, P, f], _F32, kind="ExternalInput")
    ns = nc.dram_tensor("ns", [tiles, P, f], _F32, kind="ExternalInput")
    # Partial-sum outputs: host finishes the reduction.
    # rows: 0=sum_nx, 1=sum_ny, 2=sum_nxy, 3=sum_n, 4=sum_nxx
    o_stats = nc.dram_tensor("o_stats", [6, P, tiles], _F32, kind="ExternalOutput")
    o_diag = nc.dram_tensor("o_diag", [P, P], _F32, kind="ExternalOutput")
    # variant 1: sum_nxx comes from a second PE diag pair instead of
    # GPSIMD mul + ACT accum; its diagonal lands in o_diag2.
    o_diag2 = nc.dram_tensor("o_diag2", [P, P], _F32, kind="ExternalOutput")
    # Tiny passthrough (tick->tock) so a bench harness can chain executions
    # with a data dependency; costs two 4KB DMAs.
    tick = nc.dram_tensor("tick", [P, 8], _F32, kind="ExternalInput")
    tock = nc.dram_tensor("tock", [P, 8], _F32, kind="ExternalOutput")

    with tile.TileContext(nc) as tc:
        with (
            tc.tile_pool(name="ins", bufs=in_bufs) as inp,
            tc.tile_pool(name="prods", bufs=prod_bufs) as prods,
            tc.tile_pool(name="acc", bufs=1) as accp,
            tc.tile_pool(name="psum", bufs=1, space="PSUM") as psump,
        ):
            stats_x = accp.tile([P, tiles], _F32, tag="sx")
            stats_y = accp.tile([P, tiles], _F32, tag="sy")
            stats_xy = accp.tile([P, tiles], _F32, tag="sxy")
            stats_n = accp.tile([P, tiles], _F32, tag="sn")
            stats_xx = accp.tile([P, tiles], _F32, tag="sxx")
            stats_yy2 = accp.tile([P, tiles], _F32, tag="syy2")
            if variant not in (0, 3):
                nc.vector.memset(stats_xx[:], 0.0)
            if variant != 2:
                nc.vector.memset(stats_yy2[:], 0.0)

            psum_yy = psump.tile([P, P], _F32, tag="pyy")
            psum_xx = psump.tile([P, P], _F32, tag="pxx")

            n_iter = rounds * tiles
            for rt in range(n_iter):
                t = rt % tiles
                x_t = inp.tile([P, f], _F32, tag="x")
                y_t = inp.tile([P, f], _F32, tag="y")
                n_t = inp.tile([P, f], _F32, tag="n")
                if variant == 3:
                    h = f // 2
                    for src, dst in ((ys, y_t), (ns, n_t), (xs, x_t)):
                        nc.sync.dma_start(out=dst[:, 0:h], in_=src[t][:, 0:h])
                        nc.sync.dma_start(out=dst[:, h:f], in_=src[t][:, h:f])
                else:
                    nc.sync.dma_start(out=x_t[:], in_=xs[t])
                    nc.sync.dma_start(out=y_t[:], in_=ys[t])
                    nc.sync.dma_start(out=n_t[:], in_=ns[t])

                nx_t = prods.tile([P, f], _F32, tag="nx")
                ny_t = prods.tile([P, f], _F32, tag="ny")
                junk_t = prods.tile([P, f], _F32, tag="junk")
                nxx_t = prods.tile([P, f], _F32, tag="nxx")
                ajunk_t = prods.tile([P, f], _F32, tag="ajunk")
                ajunk2_t = prods.tile([P, f], _F32, tag="ajunk2")

                # DVE: products + fused free-axis sums (custom-DVE op:
                # out = (in0*1+0)*in1, accum_out = sum(out)).
                # ny first: it feeds the PE diag matmuls, shortening PE's
                # per-tile idle gap (HAM re-throttle avoidance).
                nc.vector.affine_mul_reduce(
                    out=ny_t[:], accum_out=stats_y[:, t : t + 1],
                    in0=y_t[:], in1=n_t[:], scale=1.0, bias=0.0,
                )
                nc.vector.affine_mul_reduce(
                    out=nx_t[:], accum_out=stats_x[:, t : t + 1],
                    in0=x_t[:], in1=n_t[:], scale=1.0, bias=0.0,
                )
                nc.vector.affine_mul_reduce(
                    out=junk_t[:], accum_out=stats_xy[:, t : t + 1],
                    in0=nx_t[:], in1=y_t[:], scale=1.0, bias=0.0,
                )

                if variant in (0, 3):
                    # GPSIMD: nxx = nx * x.
                    nc.gpsimd.tensor_tensor(
                        out=nxx_t[:], in0=nx_t[:], in1=x_t[:], op=_MULT
                    )
                    # ACT: free-axis sum of nxx via Copy + accumulate.
                    nc.scalar.activation(
                        out=ajunk2_t[:], in_=nxx_t[:], func=_COPY,
                        accum_out=stats_xx[:, t : t + 1],
                    )
                elif variant == 1:
                    # PE: second diag pair (nx, x) -> sum n*x^2; GPSIMD idle.
                    for c in range(c128):
                        s = bass.ts(c, 128)
                        nc.tensor.matmul(
                            psum_xx[:],
                            nx_t[:, s],
                            x_t[:, s],
                            start=(rt == 0 and c == 0),
                            stop=(rt == n_iter - 1 and c == c128 - 1),
                        )
                elif variant == 2:
                    # Like variant 0, plus: offload 1/4 of the (ny,y) diag
                    # columns from PE to GPSIMD mul + ACT accum.
                    split = f // 4
                    nc.gpsimd.tensor_tensor(
                        out=nxx_t[:], in0=nx_t[:], in1=x_t[:], op=_MULT
                    )
                    nc.scalar.activation(
                        out=ajunk2_t[:], in_=nxx_t[:], func=_COPY,
                        accum_out=stats_xx[:, t : t + 1],
                    )
                    nyy_t = prods.tile([P, f // 4], _F32, tag="nyy")
                    ajunk3_t = prods.tile([P, f // 4], _F32, tag="ajunk3")
                    nc.gpsimd.tensor_tensor(
                        out=nyy_t[:], in0=ny_t[:, 0:split], in1=y_t[:, 0:split],
                        op=_MULT,
                    )
                    nc.scalar.activation(
                        out=ajunk3_t[:], in_=nyy_t[:], func=_COPY,
                        accum_out=stats_yy2[:, t : t + 1],
                    )

                # ACT: free-axis sum of n via Copy + accumulate.
                nc.scalar.activation(
                    out=ajunk_t[:], in_=n_t[:], func=_COPY,
                    accum_out=stats_n[:, t : t + 1],
                )

                # PE: fp32 diag-matmuls -> sum over chunks of ny_chunk^T @ y_chunk;
                # only the accumulated diagonal is meaningful (= sum n*y^2).
                c_lo = (c128 // 4) if variant == 2 else 0
                for c in range(c_lo, c128):
                    s = bass.ts(c, 128)
                    nc.tensor.matmul(
                        psum_yy[:],
                        ny_t[:, s],
                        y_t[:, s],
                        start=(rt == 0 and c == c_lo),
                        stop=(rt == n_iter - 1 and c == c128 - 1),
                    )

            nc.sync.dma_start(out=o_stats[0], in_=stats_x[:])
            nc.sync.dma_start(out=o_stats[1], in_=stats_y[:])
            nc.sync.dma_start(out=o_stats[2], in_=stats_xy[:])
            nc.sync.dma_start(out=o_stats[3], in_=stats_n[:])
            nc.sync.dma_start(out=o_stats[4], in_=stats_xx[:])
            nc.sync.dma_start(out=o_stats[5], in_=stats_yy2[:])
            # DMA cannot read PSUM: bounce through SBUF.
            sb_yy = accp.tile([P, P], _F32, tag="sbyy")
            nc.vector.tensor_copy(out=sb_yy[:], in_=psum_yy[:])
            nc.sync.dma_start(out=o_diag[:], in_=sb_yy[:])
            sb_xx = accp.tile([P, P], _F32, tag="sbxx2")
            if variant == 1:
                nc.vector.tensor_copy(out=sb_xx[:], in_=psum_xx[:])
            else:
                nc.vector.memset(sb_xx[:], 0.0)
            nc.sync.dma_start(out=o_diag2[:], in_=sb_xx[:])
            tick_t = accp.tile([P, 8], _F32, tag="tick")
            nc.sync.dma_start(out=tick_t[:], in_=tick[:])
            nc.sync.dma_start(out=tock[:], in_=tick_t[:])

    nc.finalize()
    return nc


_NC_CACHE = None


def _get_nc():
    global _NC_CACHE
    if _NC_CACHE is None:
        _NC_CACHE = build_nc()
    return _NC_CACHE


def combine_partials(results):
    """Host-side all-reduce of the per-core partials + correlation formula."""
    sum_x = sum_y = sum_prod = sum_n = sum_x2 = sum_y2 = 0.0
    for r in results:
        st = np.asarray(r["o_stats"], dtype=np.float64)
        sum_x += st[0].sum()
        sum_y += st[1].sum()
        sum_prod += st[2].sum()
        sum_n += st[3].sum()
        sum_x2 += st[4].sum()
        sum_x2 += np.diag(np.asarray(r["o_diag2"], dtype=np.float64)).sum()
        sum_y2 += np.diag(np.asarray(r["o_diag"], dtype=np.float64)).sum()
        sum_y2 += st[5].sum()
    numerator = sum_n * sum_prod - sum_x * sum_y
    denominator = np.sqrt(sum_n * sum_x2 - sum_x * sum_x) * np.sqrt(
        sum_n * sum_y2 - sum_y * sum_y
    )
    return np.asarray([numerator / denominator], dtype=np.float32)


def kernel(xs, ys, ns, **run_kwargs):
    xs = np.ascontiguousarray(np.asarray(xs, dtype=np.float32)).reshape(
        N_CORES, T, P, F
    )
    ys = np.ascontiguousarray(np.asarray(ys, dtype=np.float32)).reshape(
        N_CORES, T, P, F
    )
    ns = np.ascontiguousarray(np.asarray(ns, dtype=np.float32)).reshape(
        N_CORES, T, P, F
    )
    zt = np.zeros((P, 8), dtype=np.float32)
    in_maps = [
        {"xs": xs[c], "ys": ys[c], "ns": ns[c], "tick": zt} for c in range(N_CORES)
    ]
    res = run_bass_kernel_spmd(
        _get_nc(), in_maps, core_ids=list(range(N_CORES)), **run_kwargs
    )
    return combine_partials(res.results)



# revision 2
# speedup vs baseline: 1.3251x; 1.3251x over previous
"""Weighted Pearson correlation (six fused global reductions) on 8 trn2 cores.

Sharding: data-parallel over the flat N=2^25 dimension; each core reduces its
4M-element shard to a small set of partial sums which the host combines.

Per-core engine split (targets the ~140us/core HBM roofline for the 48MiB shard):
  - DVE    : 3 fused tensor_tensor_reduce ops/tile -> products nx=n*x, ny=n*y
             plus the cancellation-sensitive sums (sum_nx, sum_ny, sum_nxy),
             all in fp32.
  - GPSIMD : 1 tensor_tensor mul/tile -> nxx = nx*x.
  - ACT    : 2 activation-Copy ops/tile with accum_out -> sum_n, sum_nxx
             (free-axis fp32 accumulation; ACT is otherwise idle).
  - PE     : fp32 "diagonal" matmuls: diag(ny_chunk^T @ y_chunk) accumulated in
             one PSUM bank across all chunks/tiles; the diagonal of the final
             128x128 block is the per-column partial of sum_ny2.
Host: gathers per-core partials (a few KB), reduces in float64, applies the
12-flop correlation formula.
"""

import numpy as np

import concourse.bass as bass
import concourse.bacc as bacc
import concourse.tile as tile
from concourse import mybir
from concourse.bass_utils import run_bass_kernel_spmd

N_TOTAL = 33554432  # 2^25
N_CORES = 8
P = 128  # SBUF partitions

# Per-core shard: 4194304 elements = T tiles of [P, F]
F = 1024
T = N_TOTAL // N_CORES // (P * F)  # 32

_F32 = mybir.dt.float32
_MULT = mybir.AluOpType.mult
_ADD = mybir.AluOpType.add
_COPY = mybir.ActivationFunctionType.Copy


def build_nc(tiles=T, free=F, in_bufs=5, prod_bufs=2, rounds=1, variant=1):
    """Build the per-core Bass program. All 8 cores run this same program on
    their own shard (inputs shaped [tiles, 128, free])."""
    f = free
    c128 = f // 128  # stationary operand is at most 128 columns

    nc = bacc.Bacc(None)
    xs = nc.dram_tensor("xs", [tiles, P, f], _F32, kind="ExternalInput")
    ys = nc.dram_tensor("ys", [tiles, P, f], _F32, kind="ExternalInput")
    ns = nc.dram_tensor("ns", [tiles, P, f], _F32, kind="ExternalInput")
    # Partial-sum outputs: host finishes the reduction.
    # rows: 0=sum_nx, 1=sum_ny, 2=sum_nxy, 3=sum_n, 4=sum_nxx
    o_stats = nc.dram_tensor("o_stats", [6, P, tiles], _F32, kind="ExternalOutput")
    o_diag = nc.dram_tensor("o_diag", [P, P], _F32, kind="ExternalOutput")
    # variant 1: sum_nxx comes from a second PE diag pair instead of
    # GPSIMD mul + ACT accum; its diagonal lands in o_diag2.
    o_diag2 = nc.dram_tensor("o_diag2", [P, P], _F32, kind="ExternalOutput")
    # Tiny passthrough (tick->tock) so a bench harness can chain executions
    # with a data dependency; costs two 4KB DMAs.
    tick = nc.dram_tensor("tick", [P, 8], _F32, kind="ExternalInput")
    tock = nc.dram_tensor("tock", [P, 8], _F32, kind="ExternalOutput")

    with tile.TileContext(nc) as tc:
        with (
            tc.tile_pool(name="ins", bufs=in_bufs) as inp,
            tc.tile_pool(name="prods", bufs=prod_bufs) as prods,
            tc.tile_pool(name="acc", bufs=1) as accp,
            tc.tile_pool(name="psum", bufs=1, space="PSUM") as psump,
        ):
            stats_x = accp.tile([P, tiles], _F32, tag="sx")
            stats_y = accp.tile([P, tiles], _F32, tag="sy")
            stats_xy = accp.tile([P, tiles], _F32, tag="sxy")
            stats_n = accp.tile([P, tiles], _F32, tag="sn")
            stats_xx = accp.tile([P, tiles], _F32, tag="sxx")
            stats_yy2 = accp.tile([P, tiles], _F32, tag="syy2")
            if variant not in (0, 3):
                nc.vector.memset(stats_xx[:], 0.0)
            if variant != 2:
                nc.vector.memset(stats_yy2[:], 0.0)

            psum_yy = psump.tile([P, P], _F32, tag="pyy")
            psum_xx = psump.tile([P, P], _F32, tag="pxx")

            n_iter = rounds * tiles
            for rt in range(n_iter):
                t = rt % tiles
                x_t = inp.tile([P, f], _F32, tag="x")
                y_t = inp.tile([P, f], _F32, tag="y")
                n_t = inp.tile([P, f], _F32, tag="n")
                if variant == 3:
                    h = f // 2
                    for src, dst in ((ys, y_t), (ns, n_t), (xs, x_t)):
                        nc.sync.dma_start(out=dst[:, 0:h], in_=src[t][:, 0:h])
                        nc.sync.dma_start(out=dst[:, h:f], in_=src[t][:, h:f])
                else:
                    nc.sync.dma_start(out=x_t[:], in_=xs[t])
                    nc.sync.dma_start(out=y_t[:], in_=ys[t])
                    nc.sync.dma_start(out=n_t[:], in_=ns[t])

                nx_t = prods.tile([P, f], _F32, tag="nx")
                ny_t = prods.tile([P, f], _F32, tag="ny")
                junk_t = prods.tile([P, f], _F32, tag="junk")
                nxx_t = prods.tile([P, f], _F32, tag="nxx")
                ajunk_t = prods.tile([P, f], _F32, tag="ajunk")
                ajunk2_t = prods.tile([P, f], _F32, tag="ajunk2")

                # DVE: products + fused free-axis sums (custom-DVE op:
                # out = (in0*1+0)*in1, accum_out = sum(out)).
                # ny first: it feeds the PE diag matmuls, shortening PE's
                # per-tile idle gap (HAM re-throttle avoidance).
                nc.vector.affine_mul_reduce(
                    out=ny_t[:], accum_out=stats_y[:, t : t + 1],
                    in0=y_t[:], in1=n_t[:], scale=1.0, bias=0.0,
                )
                nc.vector.affine_mul_reduce(
                    out=nx_t[:], accum_out=stats_x[:, t : t + 1],
                    in0=x_t[:], in1=n_t[:], scale=1.0, bias=0.0,
                )
                nc.vector.affine_mul_reduce(
                    out=junk_t[:], accum_out=stats_xy[:, t : t + 1],
                    in0=nx_t[:], in1=y_t[:], scale=1.0, bias=0.0,
                )

                if variant in (0, 3):
                    # GPSIMD: nxx = nx * x.
                    nc.gpsimd.tensor_tensor(
                        out=nxx_t[:], in0=nx_t[:], in1=x_t[:], op=_MULT
                    )
                    # ACT: free-axis sum of nxx via Copy + accumulate.
                    nc.scalar.activation(
                        out=ajunk2_t[:], in_=nxx_t[:], func=_COPY,
                        accum_out=stats_xx[:, t : t + 1],
                    )
                elif variant == 1:
                    # PE: second diag pair (nx, x) -> sum n*x^2; GPSIMD idle.
                    for c in range(c128):
                        s = bass.ts(c, 128)
                        nc.tensor.matmul(
                            psum_xx[:],
                            nx_t[:, s],
                            x_t[:, s],
                            start=(rt == 0 and c == 0),
                            stop=(rt == n_iter - 1 and c == c128 - 1),
                        )
                elif variant == 2:
                    # Like variant 0, plus: offload 1/4 of the (ny,y) diag
                    # columns from PE to GPSIMD mul + ACT accum.
                    split = f // 4
                    nc.gpsimd.tensor_tensor(
                        out=nxx_t[:], in0=nx_t[:], in1=x_t[:], op=_MULT
                    )
                    nc.scalar.activation(
                        out=ajunk2_t[:], in_=nxx_t[:], func=_COPY,
                        accum_out=stats_xx[:, t : t + 1],
                    )
                    nyy_t = prods.tile([P, f // 4], _F32, tag="nyy")
                    ajunk3_t = prods.tile([P, f // 4], _F32, tag="ajunk3")
                    nc.gpsimd.tensor_tensor(
                        out=nyy_t[:], in0=ny_t[:, 0:split], in1=y_t[:, 0:split],
                        op=_MULT,
                    )
                    nc.scalar.activation(
                        out=ajunk3_t[:], in_=nyy_t[:], func=_COPY,
                        accum_out=stats_yy2[:, t : t + 1],
                    )

                # ACT: free-axis sum of n via Copy + accumulate.
                nc.scalar.activation(
                    out=ajunk_t[:], in_=n_t[:], func=_COPY,
                    accum_out=stats_n[:, t : t + 1],
                )

                # PE: fp32 diag-matmuls -> sum over chunks of ny_chunk^T @ y_chunk;
                # only the accumulated diagonal is meaningful (= sum n*y^2).
                c_lo = (c128 // 4) if variant == 2 else 0
                for c in range(c_lo, c128):
                    s = bass.ts(c, 128)
                    nc.tensor.matmul(
                        psum_yy[:],
                        ny_t[:, s],
                        y_t[:, s],
                        start=(rt == 0 and c == c_lo),
                        stop=(rt == n_iter - 1 and c == c128 - 1),
                    )

            nc.sync.dma_start(out=o_stats[0], in_=stats_x[:])
            nc.sync.dma_start(out=o_stats[1], in_=stats_y[:])
            nc.sync.dma_start(out=o_stats[2], in_=stats_xy[:])
            nc.sync.dma_start(out=o_stats[3], in_=stats_n[:])
            nc.sync.dma_start(out=o_stats[4], in_=stats_xx[:])
            nc.sync.dma_start(out=o_stats[5], in_=stats_yy2[:])
            # DMA cannot read PSUM: bounce through SBUF.
            sb_yy = accp.tile([P, P], _F32, tag="sbyy")
            nc.vector.tensor_copy(out=sb_yy[:], in_=psum_yy[:])
            nc.sync.dma_start(out=o_diag[:], in_=sb_yy[:])
            sb_xx = accp.tile([P, P], _F32, tag="sbxx2")
            if variant == 1:
                nc.vector.tensor_copy(out=sb_xx[:], in_=psum_xx[:])
            else:
                nc.vector.memset(sb_xx[:], 0.0)
            nc.sync.dma_start(out=o_diag2[:], in_=sb_xx[:])
            tick_t = accp.tile([P, 8], _F32, tag="tick")
            nc.sync.dma_start(out=tick_t[:], in_=tick[:])
            nc.sync.dma_start(out=tock[:], in_=tick_t[:])

    nc.finalize()
    return nc


_NC_CACHE = None


def _get_nc():
    global _NC_CACHE
    if _NC_CACHE is None:
        _NC_CACHE = build_nc()
    return _NC_CACHE


def combine_partials(results):
    """Host-side all-reduce of the per-core partials + correlation formula."""
    sum_x = sum_y = sum_prod = sum_n = sum_x2 = sum_y2 = 0.0
    for r in results:
        st = np.asarray(r["o_stats"], dtype=np.float64)
        sum_x += st[0].sum()
        sum_y += st[1].sum()
        sum_prod += st[2].sum()
        sum_n += st[3].sum()
        sum_x2 += st[4].sum()
        sum_x2 += np.diag(np.asarray(r["o_diag2"], dtype=np.float64)).sum()
        sum_y2 += np.diag(np.asarray(r["o_diag"], dtype=np.float64)).sum()
        sum_y2 += st[5].sum()
    numerator = sum_n * sum_prod - sum_x * sum_y
    denominator = np.sqrt(sum_n * sum_x2 - sum_x * sum_x) * np.sqrt(
        sum_n * sum_y2 - sum_y * sum_y
    )
    return np.asarray([numerator / denominator], dtype=np.float32)


def kernel(xs, ys, ns, **run_kwargs):
    xs = np.ascontiguousarray(np.asarray(xs, dtype=np.float32)).reshape(
        N_CORES, T, P, F
    )
    ys = np.ascontiguousarray(np.asarray(ys, dtype=np.float32)).reshape(
        N_CORES, T, P, F
    )
    ns = np.ascontiguousarray(np.asarray(ns, dtype=np.float32)).reshape(
        N_CORES, T, P, F
    )
    zt = np.zeros((P, 8), dtype=np.float32)
    in_maps = [
        {"xs": xs[c], "ys": ys[c], "ns": ns[c], "tick": zt} for c in range(N_CORES)
    ]
    res = run_bass_kernel_spmd(
        _get_nc(), in_maps, core_ids=list(range(N_CORES)), **run_kwargs
    )
    return combine_partials(res.results)



# revision 5
# speedup vs baseline: 2.3106x; 1.7437x over previous
"""Weighted Pearson correlation (six fused global reductions) on 8 trn2 cores.

Sharding: data-parallel over the flat N=2^25 dimension; each core reduces its
4M-element shard to a small set of partial sums which the host combines.

Per-core engine split (variant=1 default; ~125-145us/core for the 48MiB shard,
vs the ~140us HBM-stack roofline / ~116us SBUF-fabric ceiling):
  - DVE    : 3 fused affine_mul_reduce ops/tile -> products nx=n*x, ny=n*y
             plus the cancellation-sensitive sums (sum_nx, sum_ny, sum_nxy),
             all in fp32.
  - ACT    : 1 activation-Copy op/tile with accum_out -> sum_n.
  - PE     : 2 fp32 "diagonal" diag-matmul chains: diag(ny_c^T @ y_c) and
             diag(nx_c^T @ x_c) accumulated in two PSUM banks across all
             chunks/tiles -> per-column partials of sum_ny2 / sum_nx2.
  - GPSIMD : idle. Keeping it off the SBUF port it shares with DVE is worth
             ~34us/pass vs the old variant-3 split (GPSIMD mul + ACT accum
             for sum_nxx): any GpSimd op serializes against DVE's 2-port
             ops on the shared-port exclusive lock.
Host: gathers per-core partials (a few KB), reduces in float64, applies the
12-flop correlation formula.
"""

import numpy as np

import concourse.bass as bass
import concourse.bacc as bacc
import concourse.tile as tile
from concourse import mybir
from concourse.bass_utils import run_bass_kernel_spmd

N_TOTAL = 33554432  # 2^25
N_CORES = 8
P = 128  # SBUF partitions

# Per-core shard: 4194304 elements = T tiles of [P, F]
F = 1024
T = N_TOTAL // N_CORES // (P * F)  # 32

_F32 = mybir.dt.float32
_MULT = mybir.AluOpType.mult
_ADD = mybir.AluOpType.add
_COPY = mybir.ActivationFunctionType.Copy


def build_nc(tiles=T, free=F, in_bufs=8, prod_bufs=2, rounds=1, variant=1):
    """Build the per-core Bass program. All 8 cores run this same program on
    their own shard (inputs shaped [tiles, 128, free])."""
    f = free
    c128 = f // 128  # stationary operand is at most 128 columns

    nc = bacc.Bacc(None)
    xs = nc.dram_tensor("xs", [tiles, P, f], _F32, kind="ExternalInput")
    ys = nc.dram_tensor("ys", [tiles, P, f], _F32, kind="ExternalInput")
    ns = nc.dram_tensor("ns", [tiles, P, f], _F32, kind="ExternalInput")
    # Partial-sum outputs: host finishes the reduction.
    # rows: 0=sum_nx, 1=sum_ny, 2=sum_nxy, 3=sum_n, 4=sum_nxx
    o_stats = nc.dram_tensor("o_stats", [6, P, tiles], _F32, kind="ExternalOutput")
    o_diag = nc.dram_tensor("o_diag", [P, P], _F32, kind="ExternalOutput")
    # variant 1: sum_nxx comes from a second PE diag pair instead of
    # GPSIMD mul + ACT accum; its diagonal lands in o_diag2.
    o_diag2 = nc.dram_tensor("o_diag2", [P, P], _F32, kind="ExternalOutput")
    # Tiny passthrough (tick->tock) so a bench harness can chain executions
    # with a data dependency; costs two 4KB DMAs.
    tick = nc.dram_tensor("tick", [P, 8], _F32, kind="ExternalInput")
    tock = nc.dram_tensor("tock", [P, 8], _F32, kind="ExternalOutput")

    with tile.TileContext(nc) as tc:
        with (
            tc.tile_pool(name="ins", bufs=in_bufs) as inp,
            tc.tile_pool(name="prods", bufs=prod_bufs) as prods,
            tc.tile_pool(name="acc", bufs=1) as accp,
            tc.tile_pool(name="psum", bufs=1, space="PSUM") as psump,
        ):
            stats_x = accp.tile([P, tiles], _F32, tag="sx")
            stats_y = accp.tile([P, tiles], _F32, tag="sy")
            stats_xy = accp.tile([P, tiles], _F32, tag="sxy")
            stats_n = accp.tile([P, tiles], _F32, tag="sn")
            stats_xx = accp.tile([P, tiles], _F32, tag="sxx")
            stats_yy2 = accp.tile([P, tiles], _F32, tag="syy2")
            if variant not in (0, 3):
                nc.vector.memset(stats_xx[:], 0.0)
            if variant != 2:
                nc.vector.memset(stats_yy2[:], 0.0)

            psum_yy = psump.tile([P, P], _F32, tag="pyy")
            psum_xx = psump.tile([P, P], _F32, tag="pxx")

            n_iter = rounds * tiles
            for rt in range(n_iter):
                t = rt % tiles
                x_t = inp.tile([P, f], _F32, tag="x")
                y_t = inp.tile([P, f], _F32, tag="y")
                n_t = inp.tile([P, f], _F32, tag="n")
                if variant == 3:
                    h = f // 2
                    for src, dst in ((ys, y_t), (ns, n_t), (xs, x_t)):
                        nc.sync.dma_start(out=dst[:, 0:h], in_=src[t][:, 0:h])
                        nc.sync.dma_start(out=dst[:, h:f], in_=src[t][:, h:f])
                else:
                    # y and n first: the first DVE op (ny) consumes them.
                    nc.sync.dma_start(out=y_t[:], in_=ys[t])
                    nc.sync.dma_start(out=n_t[:], in_=ns[t])
                    nc.sync.dma_start(out=x_t[:], in_=xs[t])

                nx_t = prods.tile([P, f], _F32, tag="nx")
                ny_t = prods.tile([P, f], _F32, tag="ny")
                junk_t = prods.tile([P, f], _F32, tag="junk")
                ajunk_t = prods.tile([P, f], _F32, tag="ajunk")
                if variant in (0, 2, 3):
                    nxx_t = prods.tile([P, f], _F32, tag="nxx")
                    ajunk2_t = prods.tile([P, f], _F32, tag="ajunk2")

                # DVE: products + fused free-axis sums (custom-DVE op:
                # out = (in0*1+0)*in1, accum_out = sum(out)).
                # ny first: it feeds the PE diag matmuls, shortening PE's
                # per-tile idle gap (HAM re-throttle avoidance).
                nc.vector.affine_mul_reduce(
                    out=ny_t[:], accum_out=stats_y[:, t : t + 1],
                    in0=y_t[:], in1=n_t[:], scale=1.0, bias=0.0,
                )
                nc.vector.affine_mul_reduce(
                    out=nx_t[:], accum_out=stats_x[:, t : t + 1],
                    in0=x_t[:], in1=n_t[:], scale=1.0, bias=0.0,
                )
                nc.vector.affine_mul_reduce(
                    out=junk_t[:], accum_out=stats_xy[:, t : t + 1],
                    in0=nx_t[:], in1=y_t[:], scale=1.0, bias=0.0,
                )

                if variant in (0, 3):
                    # GPSIMD: nxx = nx * x.
                    nc.gpsimd.tensor_tensor(
                        out=nxx_t[:], in0=nx_t[:], in1=x_t[:], op=_MULT
                    )
                    # ACT: free-axis sum of nxx via Copy + accumulate.
                    nc.scalar.activation(
                        out=ajunk2_t[:], in_=nxx_t[:], func=_COPY,
                        accum_out=stats_xx[:, t : t + 1],
                    )
                elif variant == 1:
                    # PE: second diag pair (nx, x) -> sum n*x^2; GPSIMD idle.
                    for c in range(c128):
                        s = bass.ts(c, 128)
                        nc.tensor.matmul(
                            psum_xx[:],
                            nx_t[:, s],
                            x_t[:, s],
                            start=(rt == 0 and c == 0),
                            stop=(rt == n_iter - 1 and c == c128 - 1),
                        )
                elif variant == 2:
                    # Like variant 0, plus: offload 1/4 of the (ny,y) diag
                    # columns from PE to GPSIMD mul + ACT accum.
                    split = f // 4
                    nc.gpsimd.tensor_tensor(
                        out=nxx_t[:], in0=nx_t[:], in1=x_t[:], op=_MULT
                    )
                    nc.scalar.activation(
                        out=ajunk2_t[:], in_=nxx_t[:], func=_COPY,
                        accum_out=stats_xx[:, t : t + 1],
                    )
                    nyy_t = prods.tile([P, f // 4], _F32, tag="nyy")
                    ajunk3_t = prods.tile([P, f // 4], _F32, tag="ajunk3")
                    nc.gpsimd.tensor_tensor(
                        out=nyy_t[:], in0=ny_t[:, 0:split], in1=y_t[:, 0:split],
                        op=_MULT,
                    )
                    nc.scalar.activation(
                        out=ajunk3_t[:], in_=nyy_t[:], func=_COPY,
                        accum_out=stats_yy2[:, t : t + 1],
                    )

                # ACT: free-axis sum of n via Copy + accumulate.
                nc.scalar.activation(
                    out=ajunk_t[:], in_=n_t[:], func=_COPY,
                    accum_out=stats_n[:, t : t + 1],
                )

                # PE: fp32 diag-matmuls -> sum over chunks of ny_chunk^T @ y_chunk;
                # only the accumulated diagonal is meaningful (= sum n*y^2).
                c_lo = (c128 // 4) if variant == 2 else 0
                for c in range(c_lo, c128):
                    s = bass.ts(c, 128)
                    nc.tensor.matmul(
                        psum_yy[:],
                        ny_t[:, s],
                        y_t[:, s],
                        start=(rt == 0 and c == c_lo),
                        stop=(rt == n_iter - 1 and c == c128 - 1),
                    )

            nc.sync.dma_start(out=o_stats[0], in_=stats_x[:])
            nc.sync.dma_start(out=o_stats[1], in_=stats_y[:])
            nc.sync.dma_start(out=o_stats[2], in_=stats_xy[:])
            nc.sync.dma_start(out=o_stats[3], in_=stats_n[:])
            nc.sync.dma_start(out=o_stats[4], in_=stats_xx[:])
            nc.sync.dma_start(out=o_stats[5], in_=stats_yy2[:])
            # DMA cannot read PSUM: bounce through SBUF.
            sb_yy = accp.tile([P, P], _F32, tag="sbyy")
            nc.vector.tensor_copy(out=sb_yy[:], in_=psum_yy[:])
            nc.sync.dma_start(out=o_diag[:], in_=sb_yy[:])
            sb_xx = accp.tile([P, P], _F32, tag="sbxx2")
            if variant == 1:
                nc.vector.tensor_copy(out=sb_xx[:], in_=psum_xx[:])
            else:
                nc.vector.memset(sb_xx[:], 0.0)
            nc.sync.dma_start(out=o_diag2[:], in_=sb_xx[:])
            tick_t = accp.tile([P, 8], _F32, tag="tick")
            nc.sync.dma_start(out=tick_t[:], in_=tick[:])
            nc.sync.dma_start(out=tock[:], in_=tick_t[:])

    nc.finalize()
    return nc


_NC_CACHE = None


def _get_nc():
    global _NC_CACHE
    if _NC_CACHE is None:
        _NC_CACHE = build_nc()
    return _NC_CACHE


def combine_partials(results):
    """Host-side all-reduce of the per-core partials + correlation formula."""
    sum_x = sum_y = sum_prod = sum_n = sum_x2 = sum_y2 = 0.0
    for r in results:
        st = np.asarray(r["o_stats"], dtype=np.float64)
        sum_x += st[0].sum()
        sum_y += st[1].sum()
        sum_prod += st[2].sum()
        sum_n += st[3].sum()
        sum_x2 += st[4].sum()
        sum_x2 += np.diag(np.asarray(r["o_diag2"], dtype=np.float64)).sum()
        sum_y2 += np.diag(np.asarray(r["o_diag"], dtype=np.float64)).sum()
        sum_y2 += st[5].sum()
    numerator = sum_n * sum_prod - sum_x * sum_y
    denominator = np.sqrt(sum_n * sum_x2 - sum_x * sum_x) * np.sqrt(
        sum_n * sum_y2 - sum_y * sum_y
    )
    return np.asarray([numerator / denominator], dtype=np.float32)


def kernel(xs, ys, ns, **run_kwargs):
    xs = np.ascontiguousarray(np.asarray(xs, dtype=np.float32)).reshape(
        N_CORES, T, P, F
    )
    ys = np.ascontiguousarray(np.asarray(ys, dtype=np.float32)).reshape(
        N_CORES, T, P, F
    )
    ns = np.ascontiguousarray(np.asarray(ns, dtype=np.float32)).reshape(
        N_CORES, T, P, F
    )
    zt = np.zeros((P, 8), dtype=np.float32)
    in_maps = [
        {"xs": xs[c], "ys": ys[c], "ns": ns[c], "tick": zt} for c in range(N_CORES)
    ]
    res = run_bass_kernel_spmd(
        _get_nc(), in_maps, core_ids=list(range(N_CORES)), **run_kwargs
    )
    return combine_partials(res.results)

